# revision 33
# baseline (speedup 1.0000x reference)
"""Causal multi-head attention on 8 Trainium2 NeuronCores.

Sharding: core c handles batch b = c//2 and head-half hg = c%2 (8 of 16
heads, as 4 pairs). Per core: QKV projection (bf16 matmuls, f32 PSUM),
flash-style causal attention in transposed layout (scores_T[t, s], softmax
denominator replicated into PSUM partitions 64:128 via 64 ones-columns
appended to V so normalize runs entirely on VectorE), pairwise AllGather
of the normalized attention outputs, and a column-parallel output
projection (w_o columns sharded host-side per core parity). Host
reassembles y from the per-core [m_half, s] transposed outputs.

Loop structure: s-tiles outer; each s-tile's attention is interleaved with
the next s-tile's QKV-projection matmuls and the previous s-tile's output
projection so TensorE stays dense while ScalarE runs the exps. Collectives
live alone on the gpsimd queue (the issuing queue blocks until the
collective completes, so nothing else may share it); the last s-tile uses
two 2-pair gathers and an A/B-split final output projection so the tail
exposes only one small gather latency.
"""
import sys

sys.path.insert(0, "/opt/trn_rl_repo")

import numpy as np
import ml_dtypes

import concourse.bass as bass
import concourse.mybir as mybir
import concourse.tile as tile
from concourse import bacc
from concourse.bass_utils import run_bass_kernel_spmd

BF16 = ml_dtypes.bfloat16
DT = mybir.dt.bfloat16
F32 = mybir.dt.float32
EXP = mybir.ActivationFunctionType.Exp
RECIP = mybir.ActivationFunctionType.Reciprocal

B, S, DM, H, DK = 4, 2048, 1024, 16, 64
N_CORES = 8
N_PAIRS = 4          # head pairs per core (8 heads)
N_MCH = DM // 128    # m-chunks of the model dim (contraction for QKV proj)
REPLICA_GROUPS = [[0, 1], [2, 3], [4, 5], [6, 7]]


def build_nc(seq=S, n_pairs=N_PAIRS, debug_taps=False):
    """Build the SPMD kernel graph. seq must be a multiple of 512."""
    nst = seq // 512          # 512-wide s-tiles
    ntt_all = seq // 128      # 128-wide t-tiles
    nc = bacc.Bacc("TRN2", target_bir_lowering=False, debug=False,
                   num_devices=N_CORES)

    xT = nc.dram_tensor("xT", [DM, seq], DT, kind="ExternalInput")
    wq = nc.dram_tensor("wq", [DM, 128 * n_pairs], DT, kind="ExternalInput")
    wk = nc.dram_tensor("wk", [DM, 128 * n_pairs], DT, kind="ExternalInput")
    wv = nc.dram_tensor("wv", [DM, 128 * n_pairs], DT, kind="ExternalInput")
    wo = nc.dram_tensor("wo", [2 * 128 * n_pairs, 512], DT, kind="ExternalInput")
    mask128 = nc.dram_tensor("mask128", [128, 128], DT, kind="ExternalInput")
    yT = nc.dram_tensor("yT", [512, seq], F32, kind="ExternalOutput")

    n_dch = 2 * n_pairs   # d-chunks of 128 in the gathered attention
    hw = 128 * n_pairs    # head-dim columns per core (2*n_pairs heads x 64)

    with tile.TileContext(nc) as tc:
        with (
            tc.tile_pool(name="dram", bufs=1, space="DRAM") as dram,
            tc.tile_pool(name="persist", bufs=1) as persist,
            tc.tile_pool(name="psum_p", bufs=1, space="PSUM") as pp,
            tc.tile_pool(name="psum_s", bufs=2, space="PSUM") as ps_s,
            tc.tile_pool(name="psum_av", bufs=3, space="PSUM") as ps_av,
            tc.tile_pool(name="pt", bufs=6) as p_pool,
            tc.tile_pool(name="nrm", bufs=2) as nrm,
            tc.tile_pool(name="yc", bufs=2) as ycp,
            tc.tile_pool(name="stg", bufs=2) as stg,
            tc.tile_pool(name="afp", bufs=2) as afp,
        ):
            # one distinct dram tensor per collective: sharing a tile
            # across collectives creates tile-granularity false deps that
            # serialize consumers of gather N behind gather N+1
            ag_in = [dram.tile([2, 64, n_pairs, 512], DT, name=f"agi{i}")
                     for i in range(max(nst - 1, 1))]
            ag_out = [dram.tile([2, 2, 64, n_pairs, 512], DT,
                                name=f"ago{i}")
                      for i in range(max(nst - 1, 1))]
            # last s-tile: a 2-pair gather {p0,p1} then per-pair {p2},{p3}
            ag_inL = [dram.tile([2, 64, npx, 512], DT, name=f"agiL{i}")
                      for i, npx in enumerate((2, 1, 1))]
            ag_outL = [dram.tile([2, 2, 64, npx, 512], DT, name=f"agoL{i}")
                       for i, npx in enumerate((2, 1, 1))]

            q_sb = persist.tile([128, n_pairs, seq], DT, tag="q")
            k_sb = persist.tile([128, n_pairs, seq], DT, tag="k")
            v_sb = persist.tile([128, ntt_all, 2 * n_pairs, 128], DT, tag="v")
            af3 = []
            for p in range(n_pairs):
                t3 = persist.tile([128, 2, 512], DT, tag=f"af3_{p}")
                af3.append(t3)
            m_sb = persist.tile([128, 128], DT, tag="m")
            wo_sb = persist.tile([128, n_dch, 512], DT, tag="wo")
            wq_sb = persist.tile([128, N_MCH, hw], DT, tag="wq")
            wk_sb = persist.tile([128, N_MCH, hw], DT, tag="wk")
            wv_sb = persist.tile([128, N_MCH, hw], DT, tag="wv")
            xt = []
            for st in range(nst):
                t = persist.tile([128, N_MCH, 512], DT, tag=f"xt{st}")
                xt.append(t)
            af_tiles = {}

            xT_v = xT[:].rearrange("(c p) s -> p c s", p=128)
            wv_v = wv[:].rearrange("(c p) n -> p c n", p=128)
            wq_v = wq[:].rearrange("(c p) n -> p c n", p=128)
            wk_v = wk[:].rearrange("(c p) n -> p c n", p=128)
            # chunked loads of the first-needed tiles across the three
            # DMA-capable queues so the first projection groups start at
            # chunk-arrival pace instead of waiting for whole tiles
            nc.gpsimd.dma_start(out=m_sb[:], in_=mask128[:])
            for c in range(N_MCH):
                nc.gpsimd.dma_start(out=xt[0][:, c, :], in_=xT_v[:, c, 0:512])
                nc.sync.dma_start(out=wv_sb[:, c, :], in_=wv_v[:, c, :])
                nc.scalar.dma_start(out=wq_sb[:, c, :], in_=wq_v[:, c, :])
            for c in range(N_MCH):
                nc.sync.dma_start(out=wk_sb[:, c, :], in_=wk_v[:, c, :])
            if nst > 1:
                nc.gpsimd.dma_start(
                    out=xt[1][:], in_=xT_v[:, :, 512:1024])
            for st in range(2, nst):
                nc.sync.dma_start(
                    out=xt[st][:], in_=xT_v[:, :, st * 512:(st + 1) * 512])
            nc.scalar.dma_start(
                out=wo_sb[:], in_=wo[:].rearrange("(c p) n -> p c n", p=128))
            # PE warm-up during the input-load window: dummy matmuls on a
            # memset tile keep the HAM busy-window open (no DMA dependency).
            # The warm memset must precede the big V-ones memset on the
            # vector queue or the warm-up start is delayed behind it.
            warm = persist.tile([128, 512], DT, tag="warm")
            nc.vector.memset(warm[:], 0.0)
            nc.vector.memset(v_sb[:, :, :, 64:128], 1.0)
            for wi in range(2):
                wps = ps_s.tile([128, 2, 512], F32, tag="sc",
                                name=f"warm{wi}")
                for wj in range(12):
                    nc.tensor.matmul(
                        wps[:, wj % 2, :],
                        lhsT=warm[:, 0:128], rhs=warm[:],
                        start=True, stop=True)

            yT_v = yT[:].rearrange("(t p) s -> p t s", p=128)

            # ---- emission helpers (each returns a closure doing one
            # PE-dense psum-group; used to fill PE during attention) ----
            def vproj_group(tt):
                def go():
                    st, r = tt // 4, tt % 4
                    ps = pp.tile([128, hw], F32, tag="proj", name=f"psv{tt}")
                    for c in range(N_MCH):
                        nc.tensor.matmul(
                            ps[:],
                            lhsT=xt[st][:, c, r * 128:(r + 1) * 128],
                            rhs=wv_sb[:, c, 0:hw],
                            start=(c == 0), stop=(c == N_MCH - 1))
                    nc.any.tensor_copy(
                        v_sb[:, tt, :, 0:64],
                        ps[:].rearrange("p (h k) -> p h k", k=64))
                return go

            def qkproj_group(pair, st, which):
                def go():
                    w_sb, dst = ((wq_sb, q_sb), (wk_sb, k_sb))[which]
                    ps = pp.tile([128, 512], F32, tag="proj",
                                 name=f"psqk{pair}_{st}_{which}")
                    for c in range(N_MCH):
                        nc.tensor.matmul(
                            ps[:],
                            lhsT=w_sb[:, c, pair * 128:(pair + 1) * 128],
                            rhs=xt[st][:, c, :],
                            start=(c == 0), stop=(c == N_MCH - 1))
                    nc.any.tensor_copy(
                        dst[:, pair, st * 512:(st + 1) * 512], ps[:])
                return go

            def outproj_group(mt, st):
                def go():
                    ps = pp.tile([128, 512], F32, tag="proj",
                                 name=f"pso{mt}_{st}")
                    af_t = af_tiles[st]
                    for c in range(n_dch):
                        nc.tensor.matmul(
                            ps[:],
                            lhsT=wo_sb[:, c, mt * 128:(mt + 1) * 128],
                            rhs=af_t[:, c, :],
                            start=(c == 0), stop=(c == n_dch - 1))
                    yc = ycp.tile([128, 512], F32, tag="yc", name=f"yc{mt}_{st}")
                    nc.any.tensor_copy(yc[:], ps[:])
                    nc.sync.dma_start(
                        out=yT_v[:, mt, st * 512:(st + 1) * 512], in_=yc[:])
                return go

            def proj_groups_for_st(st):
                gs = []
                for tt in range(4 * st, 4 * st + 4):
                    gs.append(vproj_group(tt))
                for pair in range(n_pairs):
                    for which in range(2):
                        gs.append(qkproj_group(pair, st, which))
                return gs

            if debug_taps:
                dpt = nc.dram_tensor("dpt", [4, 128, 2, 512], DT,
                                     kind="ExternalOutput")
                dav = nc.dram_tensor("dav", [2, 128, 512], F32,
                                     kind="ExternalOutput")

            # ---- attention for one (pair, st), software-pipelined ----
            def attention(pair, st, filler, stage, pace):
                ntt = 4 * st + 4
                av0 = ps_av.tile([128, 512], F32, tag="av",
                                 name=f"av0_{pair}_{st}")
                av1 = ps_av.tile([128, 512], F32, tag="av",
                                 name=f"av1_{pair}_{st}")
                av = [av0, av1]
                pts = {}

                def scores_and_exp(tt):
                    ps = ps_s.tile([128, 2, 512], F32, tag="sc",
                                   name=f"sc{pair}_{st}_{tt}")
                    kk = tt - 4 * st
                    f0 = kk * 128 if kk > 0 else 0  # skip masked columns
                    for h in range(2):
                        lo = h * 64
                        nc.tensor.matmul(
                            ps[:, h, f0:512],
                            lhsT=k_sb[lo:lo + 64, pair,
                                      tt * 128:(tt + 1) * 128],
                            rhs=q_sb[lo:lo + 64, pair,
                                     st * 512 + f0:(st + 1) * 512],
                            start=True, stop=True)
                    pt = p_pool.tile([128, 2, 512], DT, tag="pt",
                                     name=f"pt{pair}_{st}_{tt}")
                    kk = tt - 4 * st
                    if kk < 0:
                        nc.scalar.activation(pt[:], ps[:], EXP, scale=0.125)
                    else:
                        # diagonal: zero the fully-masked cols, exp the rest,
                        # triangular mask on the boundary 128-col block
                        nc.scalar.activation(
                            pt[:, :, kk * 128:512],
                            ps[:, :, kk * 128:512], EXP, scale=0.125)
                        for h in range(2):
                            nc.vector.tensor_mul(
                                pt[:, h, kk * 128:(kk + 1) * 128],
                                pt[:, h, kk * 128:(kk + 1) * 128],
                                m_sb[:])
                    if debug_taps and pair == 0 and st == 0:
                        nc.sync.dma_start(out=dpt[tt], in_=pt[:])
                    pts[tt] = pt

                def pv(tt):
                    pt = pts.pop(tt)
                    kk = tt - 4 * st
                    f0 = kk * 128 if kk > 0 else 0
                    for h in range(2):
                        nc.tensor.matmul(
                            av[h][:, f0:512],
                            lhsT=v_sb[:, tt, 2 * pair + h, :],
                            rhs=pt[:, h, f0:512],
                            start=(tt == 0), stop=(tt == ntt - 1))

                for tt in range(ntt + 1):
                    if tt < ntt:
                        scores_and_exp(tt)
                    if tt > 0:
                        pv(tt - 1)
                    pace["done"] += 1
                    owed = (pace["pops"] * pace["done"]) // pace["total"] \
                        - pace["popped"]
                    while filler and owed > 0:
                        filler.pop(0)()
                        pace["popped"] += 1
                        owed -= 1

                if debug_taps and pair == 0 and st == 0:
                    avc = nrm.tile([128, 512], F32, tag="avc", name="avc0")
                    nc.vector.tensor_copy(avc[:], av[0][:])
                    nc.sync.dma_start(out=dav[0], in_=avc[:])
                # normalize: PSUM rows 64:128 hold the denominator
                # replicated by V's 64 ones-columns
                rcp = nrm.tile([64, 2, 512], F32, tag="rcp",
                               name=f"rcp{pair}_{st}")
                # all-vector: the custom-DVE recip needs SBUF input, and
                # GpSimd cannot read PSUM at all. Interleave per-head so
                # the head-0 chain finishes ~1.3us earlier.
                den = nrm.tile([64, 2, 512], F32, tag="den",
                               name=f"den{pair}_{st}")
                for h in range(2):
                    nc.vector.tensor_copy(den[:, h, :], av[h][64:128, :])
                    nc.vector.reciprocal_approx_fast(
                        rcp[:, h, :], den[:, h, :])
                    nc.vector.tensor_mul(
                        stage[:, h, pair, :],
                        av[h][0:64, :], rcp[:, h, :])

            # ---------------- main s-tile-outer schedule ----------------
            pending = proj_groups_for_st(0)
            while pending:
                pending.pop(0)()
            deferred = []
            for st in range(nst):
                last = st == nst - 1
                filler = []
                if st + 1 < nst:
                    filler += proj_groups_for_st(st + 1)
                if st >= 1:
                    # at the last s-tile keep two of the previous tile's
                    # output-projection groups out of the filler: they run
                    # at the start of the tail so PE stays busy (and the
                    # clock stays up) while the last gathers fly
                    n_fill = 0 if last else 4
                    for mt in range(n_fill):
                        filler.append(outproj_group(mt, st - 1))
                    deferred = [outproj_group(mt, st - 1)
                                for mt in range(n_fill, 4)]
                stage = stg.tile([64, 2, n_pairs, 512], DT, tag="stage",
                                 name=f"stage{st}")
                total_iters = n_pairs * (4 * st + 5)
                pace = {"total": total_iters, "done": 0,
                        "pops": len(filler), "popped": 0}
                for pair in range(n_pairs):
                    attention(pair, st, filler, stage, pace)
                    if last and pair >= 1:
                        # gathers {p0,p1}, {p2}, {p3}, each its own dram
                        # tensor, all on the gpsimd queue (it carries only
                        # collectives, so the completion wait that blocks
                        # the issuing queue cannot stall anything else)
                        grp = pair - 1
                        pids = [0, 1] if grp == 0 else [pair]
                        for pi, p in enumerate(pids):
                            for h in range(2):
                                eng = nc.scalar if (grp == 2 and h == 1) \
                                    else nc.sync
                                eng.dma_start(
                                    out=ag_inL[grp][h, :, pi, :],
                                    in_=stage[:, h, p, :])
                        nc.gpsimd.collective_compute(
                            "AllGather",
                            mybir.AluOpType.bypass,
                            replica_groups=REPLICA_GROUPS,
                            ins=[ag_inL[grp][:].opt()],
                            outs=[ag_outL[grp][:].opt()],
                        )
                        # keep the gpsimd queue clean for collective
                        # triggers; scalar still owes pair-3 exps at grp1
                        # time, so grp1 rides sync despite its ring lag
                        load_engs = {
                            0: [nc.sync] * 8,
                            1: [nc.sync] * 4,
                            2: [nc.sync, nc.scalar, nc.sync, nc.scalar],
                        }[grp]
                        li = 0
                        for pi, p in enumerate(pids):
                            for g in range(2):
                                for h in range(2):
                                    load_engs[li].dma_start(
                                        out=af3[p][h * 64:(h + 1) * 64, g, :],
                                        in_=ag_outL[grp][g, h, :, pi, :])
                                    li += 1
                while filler:
                    filler.pop(0)()
                # exchange this s-tile's attention columns
                if not last:
                    for h in range(2):
                        nc.sync.dma_start(
                            out=ag_in[st][h], in_=stage[:, h, :, :])
                    nc.gpsimd.collective_compute(
                        "AllGather",
                        mybir.AluOpType.bypass,
                        replica_groups=REPLICA_GROUPS,
                        ins=[ag_in[st][:].opt()],
                        outs=[ag_out[st][:].opt()],
                    )
                    af_t = afp.tile([128, n_dch, 512], DT, tag="af",
                                    name=f"af{st}")
                    af_tiles[st] = af_t
                    for g in range(2):
                        for h in range(2):
                            nc.sync.dma_start(
                                out=af_t[h * 64:(h + 1) * 64,
                                         g * n_pairs:(g + 1) * n_pairs, :],
                                in_=ag_out[st][g, h])

            # tail: run the deferred st2 output-projection groups first
            # (inputs long ready — keeps PE hot while gathers fly), then
            # the last s-tile's projection in phases by gather arrival:
            # phase A = pairs {0,1} chunks, then pair 2, then pair 3.
            lst = nst - 1
            for g in deferred:
                g()

            lps = {}

            def last_phase(mt, pids, first, stop, pool=None):
                if first:
                    # attention is done; reuse its freed PSUM banks
                    ps = pool.tile([128, 512], F32,
                                   tag="av" if pool is ps_av else "proj",
                                   name=f"psl{mt}")
                    lps[mt] = ps
                else:
                    ps = lps[mt]
                chunks = [(g * n_pairs + p, p, g)
                          for p in pids for g in range(2)]
                for i, (c, p, g) in enumerate(chunks):
                    nc.tensor.matmul(
                        ps[:],
                        lhsT=wo_sb[:, c, mt * 128:(mt + 1) * 128],
                        rhs=af3[p][:, g, :],
                        start=(first and i == 0),
                        stop=(stop and i == len(chunks) - 1))
                if stop:
                    del lps[mt]
                    yc = ycp.tile([128, 512], F32, tag="yc",
                                  name=f"ycl{mt}")
                    # explicit engines: vector and scalar are both idle at
                    # the tail; alternate so copies and writes pipeline
                    if mt % 2 == 0:
                        nc.vector.tensor_copy(yc[:], ps[:])
                        nc.sync.dma_start(
                            out=yT_v[:, mt, lst * 512:(lst + 1) * 512],
                            in_=yc[:])
                    else:
                        nc.scalar.activation(
                            yc[:], ps[:], mybir.ActivationFunctionType.Copy)
                        nc.scalar.dma_start(
                            out=yT_v[:, mt, lst * 512:(lst + 1) * 512],
                            in_=yc[:])

            last_phase(0, [0, 1], True, False, ps_av)
            last_phase(1, [0, 1], True, False, ps_av)
            last_phase(2, [0, 1], True, False, ps_av)
            last_phase(3, [0, 1], True, False, pp)
            for mt in range(4):
                last_phase(mt, [2], False, False)
            for mt in range(4):
                last_phase(mt, [3], False, True)

            if debug_taps:
                dq = nc.dram_tensor("dq", [128, n_pairs, seq], DT,
                                    kind="ExternalOutput")
                dk = nc.dram_tensor("dk", [128, n_pairs, seq], DT,
                                    kind="ExternalOutput")
                dv = nc.dram_tensor("dv", [128, ntt_all, 2 * n_pairs, 128],
                                    DT, kind="ExternalOutput")
                for dst, src in ((dq, q_sb), (dk, k_sb), (dv, v_sb)):
                    nc.sync.dma_start(out=dst[:], in_=src[:])
    nc.compile()
    return nc


def _make_mask128():
    p = np.arange(128)[:, None]
    f = np.arange(128)[None, :]
    return (p <= f).astype(BF16)


_NC_CACHE = {}


def _get_nc(seq=S, n_pairs=N_PAIRS):
    key = (seq, n_pairs)
    if key not in _NC_CACHE:
        _NC_CACHE[key] = build_nc(seq, n_pairs)
    return _NC_CACHE[key]


def make_in_maps(x, w_qkv, w_o):
    masks = _make_mask128()
    in_maps = []
    for c in range(N_CORES):
        b, hg = c // 2, c % 2
        heads = slice(hg * 8, hg * 8 + 8)
        in_maps.append({
            "xT": np.ascontiguousarray(x[b].T).astype(BF16),
            "wq": np.ascontiguousarray(
                w_qkv[0, heads].reshape(512, DM).T).astype(BF16),
            "wk": np.ascontiguousarray(
                w_qkv[1, heads].reshape(512, DM).T).astype(BF16),
            "wv": np.ascontiguousarray(
                w_qkv[2, heads].reshape(512, DM).T).astype(BF16),
            "wo": np.ascontiguousarray(
                w_o[hg * 512:(hg + 1) * 512, :].T).astype(BF16),
            "mask128": masks,
        })
    return in_maps


def kernel(x, w_qkv, w_o):
    x = np.asarray(x, dtype=np.float32)
    w_qkv = np.asarray(w_qkv, dtype=np.float32)
    w_o = np.asarray(w_o, dtype=np.float32)

    nc = _get_nc()
    in_maps = make_in_maps(x, w_qkv, w_o)
    res = run_bass_kernel_spmd(nc, in_maps, list(range(N_CORES)), trace=False)

    y = np.empty((B, S, DM), dtype=np.float32)
    for c in range(N_CORES):
        b, hg = c // 2, c % 2
        y[b, :, hg * 512:(hg + 1) * 512] = res.results[c]["yT"].T
    return y
